# revision 31
# baseline (speedup 1.0000x reference)
"""Trainium2 Bass kernel for nn_ExpandingLinear.

Reference computation (B=8192, F0=2048, E1=E2=256, O=1024, F2=2560):
    h1 = concat([x, relu(x[:, e1_parent] * e1_w)], 1)          # [B, 2304]
    h2 = concat([h1, relu(h1[:, e2_parent] * e2_w)], 1)        # [B, 2560]
    W  = scatter_add(zeros(O, F2), (w_rows, w_cols), w_vals)
    b  = scatter_add(zeros(O,), b_idx, b_vals)
    out = h2 @ W.T + b                                          # [B, O]

Algebraic reduction done on the host (weights only):
    relu(x * w) == w * relu(sign(w) * x) for scalar w, so every embed output
    column is (nonneg scalar) * relu(s * x[:, c]) for some source column c and
    sign s.  Folding each embed column's contribution through W gives

        out = x @ W0t + relu(S ⊙ xg) @ A + 1·bias

    where W0t = W[:, :2048].T, xg = the <=511 distinct (c, s) source columns,
    A is a small host-folded matrix, and the all-ones lhsT row adds the bias.

Device kernel (SPMD over 8 cores, batch-sharded 1024 rows/core):
    - DMA x_shard.T (16 k-tiles) + gathered sign-relu columns (RT k-tiles)
      + folded weights ((16+RT) k-tiles)
    - DVE: rt = max(sign * xg, 0); last row := 1.0 (bias row)
    - PE: out[m,n] accumulates over all k-tiles in PSUM
    - DVE: PSUM -> SBUF, DMA out
"""

import numpy as np

import concourse.bass as bass
import concourse.tile as tile
from concourse import bacc, mybir
from concourse.bass_utils import run_bass_kernel_spmd

B, F0, E1, E2, O = 8192, 2048, 256, 256, 1024
F1 = F0 + E1
F2 = F1 + E2
N_CORES = 8
BS = B // N_CORES          # 1024 batch rows per core
P = 128                    # partitions
KT_X = F0 // P             # 16 k-tiles of raw x
N_HALF = 512               # matmul moving free dim (fp32 max)

# matmul operand dtype:
#   float32  — exact, but the PE runs fp32 at 4 cycles/row (~296 us)
#   float32r — TF32 datapath, 1 cycle/row, rel err ~3e-4 (~97 us)  <== default
#   bfloat16 — 1 cycle/row + half the DMA bytes, rel err ~2e-3 (~90 us)
MATMUL_DT = mybir.dt.bfloat16

_CACHE = {}


def _fold_weights(e1_w, e2_w, w_vals, b_vals, e1_parent, e2_parent,
                  w_rows, w_cols, b_idx):
    """Host-side weight preprocessing: densify W/b and fold the two embed
    layers' contributions into (cols, signs, A) so the device computes
    out = x @ W0t + relu(sign*x[:, cols]) @ A + bias."""
    W = np.bincount(w_rows.astype(np.int64) * F2 + w_cols.astype(np.int64),
                    weights=w_vals.astype(np.float64),
                    minlength=O * F2).reshape(O, F2)
    bias = np.bincount(b_idx.astype(np.int64), weights=b_vals.astype(np.float64),
                       minlength=O)
    W0t = W[:, :F0].T          # [2048, 1024]
    W1t = W[:, F0:F1].T        # [256, 1024]  layer-1 embed rows
    W2t = W[:, F1:F2].T        # [256, 1024]  layer-2 embed rows

    # each embed column j contributes scale*relu(s*x[:, c]) with weight row w
    # accumulate per (c, s): A_map[(c, s)] += scale * w_row
    A_map = {}

    def acc(c, s, scale, wrow):
        if scale == 0.0:
            return
        key = (int(c), int(s))
        if key in A_map:
            A_map[key] = A_map[key] + scale * wrow
        else:
            A_map[key] = scale * wrow

    e1_parent = e1_parent.astype(np.int64)
    e2_parent = e2_parent.astype(np.int64)
    e1_w64 = e1_w.astype(np.float64)
    e2_w64 = e2_w.astype(np.float64)

    for j in range(E1):
        w = e1_w64[j]
        s = 1 if w >= 0 else -1
        acc(e1_parent[j], s, abs(w), W1t[j])
    for j in range(E2):
        q = e2_parent[j]
        w = e2_w64[j]
        if q < F0:
            s = 1 if w >= 0 else -1
            acc(q, s, abs(w), W2t[j])
        else:
            # refers to layer-1 embed column m1: h1e[:, m1] >= 0 always
            if w < 0:
                continue  # relu(negative * nonneg) == 0
            m1 = q - F0
            w1 = e1_w64[m1]
            s = 1 if w1 >= 0 else -1
            acc(e1_parent[m1], s, w * abs(w1), W2t[j])

    pairs = sorted(A_map.keys())
    n_pairs = len(pairs)
    # relu-block k-tiles; last row of the block is reserved for the bias row
    RT = max(1, -(-(n_pairs + 1) // P))
    n_rows = RT * P
    cols = np.zeros(n_rows, dtype=np.int64)
    signs = np.ones(n_rows, dtype=np.float32)
    A = np.zeros((n_rows, O), dtype=np.float64)
    for i, (c, s) in enumerate(pairs):
        cols[i] = c
        signs[i] = s
        A[i] = A_map[(c, s)]
    return (W0t.astype(np.float32), A.astype(np.float32),
            bias.astype(np.float32), cols, signs, RT)


def _build_program(RT):
    """Build + compile the SPMD Bass program (same for every core)."""
    KT = KT_X + RT  # total k-tiles
    MDT = MATMUL_DT
    nc = bacc.Bacc("TRN2", target_bir_lowering=False, debug=False,
                   num_devices=N_CORES)

    # fp32r relu sources stay f32 (the DVE write rounds); bf16 arrives bf16
    GDT = MDT if MDT == mybir.dt.bfloat16 else mybir.dt.float32
    xt_d = nc.dram_tensor("xt", [KT_X, P, BS], MDT, kind="ExternalInput")
    xg_d = nc.dram_tensor("xg", [RT, P, BS], GDT, kind="ExternalInput")
    wc_d = nc.dram_tensor("wc", [KT, P, O], MDT, kind="ExternalInput")
    sg_d = nc.dram_tensor("sg", [P, RT], mybir.dt.float32,
                          kind="ExternalInput")
    # [m, n, p, c] layout: each [128, 512] half-store is contiguous
    out_d = nc.dram_tensor("out", [BS // P, O // N_HALF, P, N_HALF],
                           mybir.dt.float32, kind="ExternalOutput")

    with tile.TileContext(nc) as tc:
        with (
            tc.tile_pool(name="sbuf", bufs=1) as pool,
            tc.tile_pool(name="outp", bufs=1) as outp,
            tc.tile_pool(name="psum", bufs=8, space="PSUM") as psum,
        ):
            sg_sb = pool.tile([P, RT], mybir.dt.float32, tag="sg")
            nc.sync.dma_start(sg_sb[:], sg_d[:])

            # PE warm-up: ~20 data-independent matmuls fill the otherwise
            # idle DMA-ramp window and flip the HAM clock gate to 2.4 GHz
            # before the real stream starts (cold matmuls run at 1.2 GHz)
            # tiny first transfer (128 KiB) so warm-up can start ASAP;
            # DMA producers satisfy the fp32r-rounded verifier check
            wrm = pool.tile([P, 256], MDT, tag="wrm", name="wrm")
            nc.sync.dma_start(wrm[:], wc_d[0][:, :256])
            wps = psum.tile([P, N_HALF], mybir.dt.float32, tag="ps",
                            name="wps")
            for _ in range(16):
                nc.tensor.matmul(wps[:, :256], wrm[:, :P], wrm[:],
                                 start=True, stop=True)

            # lhsT k-tiles (16 raw x + RT sign-relu) and weight k-tiles,
            # DMA'd pairwise (~1 MiB per transfer) in k order so early
            # k-tiles land first
            lh = [pool.tile([P, BS], MDT, tag=f"x{kt}", name=f"x{kt}")
                  for kt in range(KT_X)]
            wc = [pool.tile([P, O], MDT, tag=f"w{kt}", name=f"w{kt}")
                  for kt in range(KT)]
            # first k-tile arrives in quarter chunks so the PE can start on
            # (m0, n0) after ~256 KiB instead of ~1 MiB
            H = BS // 2
            for kt in range(KT):
                if kt == 0:
                    nc.sync.dma_start(wc[kt][:, :H], wc_d[kt][:, :H])
                    nc.sync.dma_start(lh[kt][:, :H], xt_d[kt][:, :H])
                    nc.sync.dma_start(wc[kt][:, H:], wc_d[kt][:, H:])
                    nc.sync.dma_start(lh[kt][:, H:], xt_d[kt][:, H:])
                else:
                    nc.sync.dma_start(wc[kt][:], wc_d[kt])
                    if kt < KT_X:
                        nc.sync.dma_start(lh[kt][:], xt_d[kt])
            # gathered relu-source columns AFTER the main stream: a g-slot
            # ring wait here would head-of-line-block the in-order sync
            # DMA queue, so these must not sit in front of any k-tile
            for t in range(RT):
                g_sb = pool.tile([P, BS], GDT, tag="g",
                                 name=f"g{t}", bufs=2)
                nc.sync.dma_start(g_sb[:], xg_d[t])
                r_sb = pool.tile([P, BS], MDT, tag=f"r{t}", name=f"r{t}")
                # bias row: xg's last row is all-ones with sign +1, so the
                # sign-relu passes it through unchanged; out dtype rounds
                # to fp32r when MDT is float32r
                nc.vector.tensor_scalar(r_sb[:], g_sb[:],
                                        sg_sb[:, t:t + 1], 0.0,
                                        mybir.AluOpType.mult,
                                        mybir.AluOpType.max)
                lh.append(r_sb)

            # K-outer waves of KC tiles: PE consumes k-tiles in DMA arrival
            # order and never waits on far-away tiles. 16 (m, n) output
            # groups > 8 PSUM banks, so each wave runs two passes of 8
            # groups (second pass re-reads the same resident k-tiles).
            # Waves accumulate into o_sb via DVE.
            MT = BS // P           # 8 m-tiles
            NT = O // N_HALF       # 2 n-halves
            groups = [(m, n) for m in range(MT) for n in range(NT)]
            o_sbs = [outp.tile([P, O], mybir.dt.float32, tag=f"o{m}",
                               name=f"o{m}") for m in range(MT)]
            # small waves while the k-stream is in flight, one big wave
            # once everything is resident (fewer DVE accumulate ops)
            waves = [(0, 4), (4, 8), (8, 12), (12, KT)]
            for wi, (k0, k1) in enumerate(waves):
                last_wave = wi == len(waves) - 1
                for half in range(2):
                    gsl = groups[half * 8:(half + 1) * 8]
                    pss = {g: psum.tile([P, N_HALF], mybir.dt.float32,
                                        tag="ps", name="ps") for g in gsl}
                    if last_wave:
                        # everything is resident by now: run group-major so
                        # each group's accumulate + store pipelines right
                        # behind its last matmul instead of all at the end
                        order = [(kt, g) for g in gsl for kt in range(k0, k1)]
                    else:
                        # k-major: consume k-tiles in DMA arrival order
                        order = [(kt, g) for kt in range(k0, k1) for g in gsl]
                    for kt, (m, n) in order:
                        nc.tensor.matmul(
                            pss[(m, n)][:],
                            lh[kt][:, m * P:(m + 1) * P],
                            wc[kt][:, n * N_HALF:(n + 1) * N_HALF],
                            start=(kt == k0),
                            stop=(kt == k1 - 1),
                        )
                        if kt != k1 - 1:
                            continue
                        # group complete: drain its PSUM bank immediately so
                        # the next half's matmuls find a free bank without
                        # waiting (DVE accumulate into the persistent o_sb)
                        osl = o_sbs[m][:, n * N_HALF:(n + 1) * N_HALF]
                        if wi == 0:
                            nc.vector.tensor_copy(osl, pss[(m, n)][:])
                        else:
                            nc.vector.tensor_add(osl, osl, pss[(m, n)][:])
                        if last_wave:
                            # alternate stores across both HW DMA queues
                            # (the input stream on sync is long done by the
                            # time these fire) to double store bandwidth
                            eng = nc.scalar if (m + n) % 2 else nc.sync
                            eng.dma_start(out_d[m][n], osl)

    nc.compile()
    return nc


def _round_tf32(a):
    """Round-to-nearest-even fp32 -> tf32 (10-bit mantissa), like the PE's
    fp32r datapath expects (low 13 mantissa bits zero)."""
    u = a.astype(np.float32).view(np.uint32)
    rb = (u >> np.uint32(13)) & np.uint32(1)
    u = (u + np.uint32(0x0FFF) + rb) & np.uint32(0xFFFFE000)
    return u.view(np.float32)


def kernel(input, e1_w, e2_w, w_vals, b_vals, e1_parent, e2_parent,
           w_rows, w_cols, b_idx):
    input = np.asarray(input, dtype=np.float32)
    W0t, A, bias, cols, signs, RT = _fold_weights(
        np.asarray(e1_w), np.asarray(e2_w), np.asarray(w_vals),
        np.asarray(b_vals), np.asarray(e1_parent), np.asarray(e2_parent),
        np.asarray(w_rows), np.asarray(w_cols), np.asarray(b_idx))

    KT = KT_X + RT
    # weight slab: [KT*128, O] = [W0t ; A-with-bias-row]
    wc = np.concatenate([W0t, A], axis=0)
    wc[KT * P - 1, :] = bias           # lhsT row is all-ones -> adds bias
    wc = np.ascontiguousarray(wc.reshape(KT, P, O), dtype=np.float32)
    sg = np.ascontiguousarray(signs.reshape(RT, P).T, dtype=np.float32)

    key = (RT, MATMUL_DT)
    if key not in _CACHE:
        _CACHE[key] = _build_program(RT)
    nc = _CACHE[key]

    xg_full = input[:, cols]           # [B, RT*128] gathered source columns
    xg_full[:, RT * P - 1] = 1.0       # all-ones bias column (sign is +1)
    xmm = input
    if MATMUL_DT == mybir.dt.float32r:
        xmm = _round_tf32(input)
        xg_full = _round_tf32(xg_full)  # relu/sign-mult commute with rounding
        wc = _round_tf32(wc)
    elif MATMUL_DT == mybir.dt.bfloat16:
        import ml_dtypes
        bf = np.dtype(ml_dtypes.bfloat16)
        xmm = input.astype(bf)
        xg_full = xg_full.astype(bf)
        wc = wc.astype(bf)
    in_maps = []
    for c in range(N_CORES):
        sl = slice(c * BS, (c + 1) * BS)
        xt_c = np.ascontiguousarray(xmm[sl].T.reshape(KT_X, P, BS))
        xg_c = np.ascontiguousarray(xg_full[sl].T.reshape(RT, P, BS))
        in_maps.append({"xt": xt_c, "xg": xg_c, "wc": wc, "sg": sg})

    res = run_bass_kernel_spmd(nc, in_maps, list(range(N_CORES)))
    out = np.concatenate(
        [res.results[c]["out"].transpose(0, 2, 1, 3).reshape(BS, O)
         for c in range(N_CORES)], axis=0)
    return out
